# revision 39
# baseline (speedup 1.0000x reference)
"""BiDense (binary dense) kernel for Trainium2, column-parallel over 8 NeuronCores.

Math (mirrors the reference exactly):
    bk[f] = max_d |kernel[d, f]| + f32_eps          (per-output-feature bound)
    bx[t] = max_d |x[t, d]|      + f32_eps          (per-token bound)
    kq = sign*(kernel) * 0.5 * bk[f]                (sign* maps 0 -> +1)
    xq = sign*(x)      * 0.5 * bx[t]
    y[t, f] = sum_d xq kq + bias[f]
            = 0.25 * bx[t] * bk[f] * (Sx @ Sk)[t, f] + bias[f]

Sx/Sk are +-1 matrices, so the GEMM runs exactly in fp8 (products are +-1,
accumulation of <=4096 integers is exact in fp32 PSUM).

Pipeline per core (f-shard):
  - x and k are host-staged to bf16 (only sign + max-abs bound are taken from
    them, so the 2^-8 bf16 rounding only perturbs the output scale ~0.4%,
    well inside the 2e-2 tolerance; signs are exactly preserved).
  - x streams twice from DRAM: once natural [t, d] for the per-token bound
    (DVE abs-max reduce), once through the DMA XBAR transpose straight into
    SBUF as [d, t] bf16.  One ACT sign pass turns the transposed tile into
    fp8e4 +-1 lhsT.  This keeps the Tensor engine free of the 2048
    transpose+evacuate pairs the previous version spent ~350us on.
  - k streams on two DMA queues, ACT computes Sk signs, DVE tracks the
    running max|k| with single abs_max ops, GPSIMD reduces across partitions.
  - PE runs only the DoubleRow fp8 matmuls (the hard roofline).
"""

import numpy as np
import ml_dtypes
from contextlib import ExitStack

import concourse.bass as bass
import concourse.bass_isa as bass_isa
import concourse.mybir as mybir
import concourse.tile as tile
from concourse import bacc, bass_utils

P = 128
N_CORES = 8
F32_EPS = float(np.finfo(np.float32).eps)
SIGN_BIAS = 1e-30  # sign(v + tiny): maps v==0 to +1, never flips a real value

FP32 = mybir.dt.float32
BF16 = mybir.dt.bfloat16
FP8 = mybir.dt.float8e4
ALU = mybir.AluOpType
AX = mybir.AxisListType
BF16NP = ml_dtypes.bfloat16


def build_nc(T, D, F, has_bias=False, NF=512, PRE=6):
    """Build the per-core Bass program.

    T: tokens (rows of x) handled by this core
    D: contraction dim
    F: features handled by this core (the f shard)
    PRE: token blocks staged ahead (x loads, transpose, sign)
    """
    assert T % P == 0 and D % P == 0 and F % NF == 0 and NF % P == 0
    KT = D // P            # contraction tiles
    TB = T // P            # token blocks
    FC = F // NF           # psum chunks along f
    assert KT % 2 == 0
    PRE = min(PRE, TB)

    nc = bacc.Bacc(trn_type="TRN2")
    x_d = nc.dram_tensor("x_in", [T, D], BF16, kind="ExternalInput")
    k_d = nc.dram_tensor("k_in", [D, F], BF16, kind="ExternalInput")
    b_d = None
    if has_bias:
        b_d = nc.dram_tensor("b_in", [F], FP32, kind="ExternalInput")
    y_d = nc.dram_tensor("y_out", [T, F], BF16, kind="ExternalOutput")

    with ExitStack() as ctx:
        tc = ctx.enter_context(tile.TileContext(nc))
        const = ctx.enter_context(tc.tile_pool(name="const", bufs=1))
        skp = ctx.enter_context(tc.tile_pool(name="sk", bufs=1))
        dramp = ctx.enter_context(tc.tile_pool(name="dram", bufs=1, space="DRAM"))
        mmps = ctx.enter_context(tc.tile_pool(name="mmps", bufs=2 * FC, space="PSUM"))
        xnp = ctx.enter_context(tc.tile_pool(name="xnp", bufs=2))
        xtp = ctx.enter_context(tc.tile_pool(name="xtp", bufs=2))
        sxtp = ctx.enter_context(tc.tile_pool(name="sxtp", bufs=PRE + 2))
        outp = ctx.enter_context(tc.tile_pool(name="outp", bufs=4))
        bxp = ctx.enter_context(tc.tile_pool(name="bxp", bufs=PRE + 4))

        sbias = const.tile([P, 1], FP32)   # tiny bias so sign(0+eps) = +1
        nc.vector.memset(sbias, SIGN_BIAS)

        sk = skp.tile([P, KT, F], FP8)            # Sk signs, [d_lo, kt, f]
        bkb = const.tile([P, F], FP32)            # 0.25*(bk+eps), bcast on parts
        biasb = (const.tile([P, F], FP32, name="biasb") if has_bias else None)
        mx2 = const.tile([P, 2, F], BF16)         # running max k (pair planes)
        mn2 = const.tile([P, 2, F], BF16)         # running min k (pair planes)
        mxf = const.tile([P, F], BF16)            # folded max|k| per partition

        def emit_k1(kload, kt):
            # one kernel-stream step, two contraction tiles at a time:
            # ktile[p, a, f] = k[(kt+a)*P + p, f]; ACT computes Sk signs
            # (+-1, 0 -> +1) straight into the matching sk slices; DVE keeps
            # bf16 running max/min for the per-feature bound.
            ktile = kload.tile([P, 2, F], BF16, tag="kl", name="ktile")
            nc.gpsimd.dma_start(ktile[:, 0, :], k_d[kt * P:(kt + 1) * P, :])
            nc.gpsimd.dma_start(ktile[:, 1, :], k_d[(kt + 1) * P:(kt + 2) * P, :])
            nc.scalar.sign(sk[:, kt:kt + 2, :], ktile, bias=sbias[:])
            if kt == 0:
                nc.vector.tensor_copy(mx2, ktile)
                nc.vector.tensor_copy(mn2, ktile)
            else:
                nc.vector.tensor_tensor(mx2, mx2, ktile, op=ALU.max)
                nc.vector.tensor_tensor(mn2, mn2, ktile, op=ALU.min)

        def emit_bounds_final():
            # fold: max(|mx|, |mn|) per (pair, f), then across the pair
            # planes, then across partitions on GPSIMD (broadcast result).
            nc.vector.scalar_tensor_tensor(
                mx2, mn2, -1.0, mx2, op0=ALU.mult, op1=ALU.max)
            nc.vector.tensor_tensor(mxf, mx2[:, 0, :], mx2[:, 1, :],
                                    op=ALU.max)
            nc.gpsimd.partition_all_reduce(bkb, mxf, channels=P,
                                           reduce_op=bass_isa.ReduceOp.max)
            nc.vector.tensor_scalar(bkb, bkb, F32_EPS, 0.25,
                                    op0=ALU.add, op1=ALU.mult)
            if has_bias:
                bsrc = b_d[:]
                bbcast = bass.AP(tensor=bsrc.tensor, offset=bsrc.offset,
                                 ap=[[0, P]] + [list(pair) for pair in bsrc.ap])
                nc.sync.dma_start(biasb, bbcast)

        bx_tiles = {}
        sxt_tiles = {}

        def emit_xpair(j):
            # stage token blocks j and j+1 together.
            # natural-layout stream (gpsimd queue): feeds the per-token bound
            for b in (j, j + 1):
                x_t = xnp.tile([P, D], BF16, tag="x", name="x_t")
                nc.gpsimd.dma_start(x_t, x_d[b * P:(b + 1) * P, :])
                bxq = bxp.tile([P, 1], FP32, tag="bx", name="bxq")
                nc.vector.tensor_reduce(bxq, x_t, axis=AX.X, op=ALU.max,
                                        apply_absolute_value=True)
                nc.vector.tensor_scalar_add(bxq, bxq, F32_EPS)
                bx_tiles[b] = bxq
            # transposed stream through the DMA XBAR, two token blocks per
            # instruction (amortizes the ~12.5us fixed wait each transpose
            # pays): xt[p, kt, t2] = x[jP+t2, kt*P+p] with t2 in [0, 2P).
            # The ucode transpose occupies the issuing engine for its whole
            # transfer, so keeping every transpose on the sync queue also
            # serializes the shared XBAR (two in flight corrupt each other -
            # measured).  ACT signs are split per half so the GEMM, which
            # consumes kt in order, starts on the low half.
            KH = KT // 2
            xt = xtp.tile([P, KT, 2 * P], BF16, tag="xt", name="xt")
            nc.sync.dma_start_transpose(xt, x_d[j * P:(j + 2) * P, :])
            for b in (j, j + 1):
                ts = slice((b - j) * P, (b - j + 1) * P)
                sxt = sxtp.tile([P, KT, P], FP8, tag="sxt", name="sxt")
                nc.scalar.sign(sxt[:, :KH, :], xt[:, :KH, ts], bias=sbias[:])
                nc.scalar.sign(sxt[:, KH:, :], xt[:, KH:, ts], bias=sbias[:])
                sxt_tiles[b] = sxt

        # ---- preamble: the per-feature bounds stream first (they gate the
        # first psum evacuation), block 0/1 staging (so the GEMM can start),
        # then the kernel sign stream (it gates every block's tail), then
        # the remaining lookahead blocks.
        assert PRE % 2 == 0 and TB % 2 == 0
        with tc.tile_pool(name="kload", bufs=3) as kload:
            # tiny throwaway transpose: absorbs the fixed XBAR wake-up wait
            # so block 0's real transpose starts immediately behind it
            warm = const.tile([P, 16], BF16, name="warm")
            nc.sync.dma_start_transpose(warm, x_d[0:16, 0:P])
            emit_xpair(0)
            for kt in range(0, KT, 2):
                emit_k1(kload, kt)
            emit_bounds_final()
            for j in range(2, PRE, 2):
                emit_xpair(j)

        # ---- main loop over token blocks ----------------------------------
        pm = mybir.MatmulPerfMode.DoubleRow
        for i in range(TB):
            if i + PRE < TB and (i + PRE) % 2 == 0:
                emit_xpair(i + PRE)
            sxt = sxt_tiles.pop(i)
            mm_tiles = [mmps.tile([P, NF], FP32, tag="mm", name=f"mm{fc}")
                        for fc in range(FC)]

            for kt in range(0, KT, 2):
                start = kt == 0
                stop = kt + 2 >= KT
                for fc in range(FC):
                    nc.tensor.matmul(
                        mm_tiles[fc][:],
                        lhsT=sxt[:, kt:kt + 2, :],
                        rhs=sk[:, kt:kt + 2, fc * NF:(fc + 1) * NF],
                        start=start, stop=stop, perf_mode=pm)

            bxq = bx_tiles.pop(i)
            for fc in range(FC):
                sl = slice(fc * NF, (fc + 1) * NF)
                out_c = outp.tile([P, NF], BF16, tag="out", name="out_c")
                # y = (psum * bx[t]) * (0.25*bk[f])
                nc.vector.scalar_tensor_tensor(
                    out_c, mm_tiles[fc][:], bxq, bkb[:, sl],
                    op0=ALU.mult, op1=ALU.mult)
                if has_bias:
                    nc.vector.tensor_tensor(out_c, out_c, biasb[:, sl],
                                            op=ALU.add)
                nc.gpsimd.dma_start(y_d[i * P:(i + 1) * P, sl], out_c)

    if not nc.is_finalized():
        nc.finalize()
    return nc


def _run(x2, ksh_list, bias_list, has_bias, trace=False, NF=512, PRE=6):
    """Compile once and run the SPMD program on all 8 cores."""
    T, D = x2.shape
    F = ksh_list[0].shape[1]
    nc = build_nc(T, D, F, has_bias=has_bias, NF=NF, PRE=PRE)
    x_bf = np.ascontiguousarray(np.asarray(x2).astype(BF16NP))
    in_maps = []
    for c in range(len(ksh_list)):
        k_bf = np.asarray(ksh_list[c]).astype(BF16NP)
        m = {"x_in": x_bf,
             "k_in": np.ascontiguousarray(k_bf)}
        if has_bias:
            m["b_in"] = np.ascontiguousarray(np.asarray(bias_list[c], np.float32))
        in_maps.append(m)
    res = bass_utils.run_bass_kernel_spmd(
        nc, in_maps, core_ids=list(range(len(ksh_list))), trace=trace)
    return res


def kernel(x, kernel, bias):
    x = np.asarray(x, dtype=np.float32)
    k = np.asarray(kernel, dtype=np.float32)
    b = np.asarray(bias, dtype=np.float32)
    B, S, D = x.shape
    F = k.shape[1]
    T = B * S
    FS = F // N_CORES
    x2 = np.ascontiguousarray(x.reshape(T, D))
    has_bias = bool(np.any(b))
    ksh = [np.ascontiguousarray(k[:, c * FS:(c + 1) * FS]) for c in range(N_CORES)]
    bsh = [np.ascontiguousarray(b[c * FS:(c + 1) * FS]) for c in range(N_CORES)]
    res = _run(x2, ksh, bsh, has_bias)
    y = np.concatenate(
        [np.asarray(res.results[c]["y_out"]).astype(np.float32)
         for c in range(N_CORES)], axis=1)
    return np.ascontiguousarray(y.reshape(B, S, F)).astype(np.float32)


# revision 41
# speedup vs baseline: 1.1521x; 1.1521x over previous
"""BiDense (binary dense) kernel for Trainium2, column-parallel over 8 NeuronCores.

Math (mirrors the reference exactly):
    bk[f] = max_d |kernel[d, f]| + f32_eps          (per-output-feature bound)
    bx[t] = max_d |x[t, d]|      + f32_eps          (per-token bound)
    kq = sign*(kernel) * 0.5 * bk[f]                (sign* maps 0 -> +1)
    xq = sign*(x)      * 0.5 * bx[t]
    y[t, f] = sum_d xq kq + bias[f]
            = 0.25 * bx[t] * bk[f] * (Sx @ Sk)[t, f] + bias[f]

Sx/Sk are +-1 matrices, so the GEMM runs exactly in fp8 (products are +-1,
accumulation of <=4096 integers is exact in fp32 PSUM).

Pipeline per core (f-shard):
  - x and k are host-staged to bf16 (only sign + max-abs bound are taken from
    them, so the 2^-8 bf16 rounding only perturbs the output scale ~0.4%,
    well inside the 2e-2 tolerance; signs are exactly preserved).
  - x streams twice from DRAM: once natural [t, d] for the per-token bound
    (DVE abs-max reduce), once through the DMA XBAR transpose straight into
    SBUF as [d, t] bf16.  One ACT sign pass turns the transposed tile into
    fp8e4 +-1 lhsT.  This keeps the Tensor engine free of the 2048
    transpose+evacuate pairs the previous version spent ~350us on.
  - k streams on two DMA queues, ACT computes Sk signs, DVE tracks the
    running max|k| with single abs_max ops, GPSIMD reduces across partitions.
  - PE runs only the DoubleRow fp8 matmuls (the hard roofline).
"""

import numpy as np
import ml_dtypes
from contextlib import ExitStack

import concourse.bass as bass
import concourse.bass_isa as bass_isa
import concourse.mybir as mybir
import concourse.tile as tile
from concourse import bacc, bass_utils

P = 128
N_CORES = 8
F32_EPS = float(np.finfo(np.float32).eps)
SIGN_BIAS = 1e-30  # sign(v + tiny): maps v==0 to +1, never flips a real value

FP32 = mybir.dt.float32
BF16 = mybir.dt.bfloat16
FP8 = mybir.dt.float8e4
ALU = mybir.AluOpType
AX = mybir.AxisListType
BF16NP = ml_dtypes.bfloat16


def build_nc(T, D, F, has_bias=False, NF=512, PRE=6):
    """Build the per-core Bass program.

    T: tokens (rows of x) handled by this core
    D: contraction dim
    F: features handled by this core (the f shard)
    PRE: token blocks staged ahead (x loads, transpose, sign)
    """
    assert T % P == 0 and D % P == 0 and F % NF == 0 and NF % P == 0
    KT = D // P            # contraction tiles
    TB = T // P            # token blocks
    FC = F // NF           # psum chunks along f
    assert KT % 2 == 0
    PRE = min(PRE, TB)

    nc = bacc.Bacc(trn_type="TRN2")
    x_d = nc.dram_tensor("x_in", [T, D], BF16, kind="ExternalInput")
    k_d = nc.dram_tensor("k_in", [D, F], BF16, kind="ExternalInput")
    b_d = None
    if has_bias:
        b_d = nc.dram_tensor("b_in", [F], FP32, kind="ExternalInput")
    y_d = nc.dram_tensor("y_out", [T, F], BF16, kind="ExternalOutput")

    with ExitStack() as ctx:
        tc = ctx.enter_context(tile.TileContext(nc))
        const = ctx.enter_context(tc.tile_pool(name="const", bufs=1))
        skp = ctx.enter_context(tc.tile_pool(name="sk", bufs=1))
        dramp = ctx.enter_context(tc.tile_pool(name="dram", bufs=1, space="DRAM"))
        mmps = ctx.enter_context(tc.tile_pool(name="mmps", bufs=2 * FC, space="PSUM"))
        xnp = ctx.enter_context(tc.tile_pool(name="xnp", bufs=2))
        xtp = ctx.enter_context(tc.tile_pool(name="xtp", bufs=2))
        sxtp = ctx.enter_context(tc.tile_pool(name="sxtp", bufs=PRE + 2))
        outp = ctx.enter_context(tc.tile_pool(name="outp", bufs=4))
        bxp = ctx.enter_context(tc.tile_pool(name="bxp", bufs=PRE + 4))

        sbias = const.tile([P, 1], FP32)   # tiny bias so sign(0+eps) = +1
        nc.vector.memset(sbias, SIGN_BIAS)

        sk = skp.tile([P, KT, F], FP8)            # Sk signs, [d_lo, kt, f]
        bkb = const.tile([P, F], FP32)            # 0.25*(bk+eps), bcast on parts
        biasb = (const.tile([P, F], FP32, name="biasb") if has_bias else None)
        mx2 = const.tile([P, 2, F], BF16)         # running max k (pair planes)
        mn2 = const.tile([P, 2, F], BF16)         # running min k (pair planes)
        mxf = const.tile([P, F], BF16)            # folded max|k| per partition

        k_tiles = {}

        def emit_kkick(kload, kt):
            # kernel-pair DMA, two contraction tiles per SBUF tile:
            # ktile[p, a, f] = k[(kt+a)*P + p, f].  These ride the gpsimd
            # queue AHEAD of everything else there - the kernel stream is
            # the preamble's critical path.
            ktile = kload.tile([P, 2, F], BF16, tag="kl", name="ktile")
            nc.gpsimd.dma_start(ktile[:, 0, :], k_d[kt * P:(kt + 1) * P, :])
            nc.gpsimd.dma_start(ktile[:, 1, :], k_d[(kt + 1) * P:(kt + 2) * P, :])
            k_tiles[kt] = ktile

        def emit_ksign(kt):
            # ACT computes Sk signs (+-1, 0 -> +1) straight into the
            # matching sk slices; DVE keeps bf16 running max/min planes.
            ktile = k_tiles.pop(kt)
            nc.scalar.sign(sk[:, kt:kt + 2, :], ktile, bias=sbias[:])
            if kt == 0:
                nc.vector.tensor_copy(mx2, ktile)
                nc.vector.tensor_copy(mn2, ktile)
            else:
                nc.vector.tensor_tensor(mx2, mx2, ktile, op=ALU.max)
                nc.vector.tensor_tensor(mn2, mn2, ktile, op=ALU.min)

        def emit_bounds_final():
            # fold: max(|mx|, |mn|) per (pair, f), then across the pair
            # planes, then across partitions on GPSIMD (broadcast result).
            nc.vector.scalar_tensor_tensor(
                mx2, mn2, -1.0, mx2, op0=ALU.mult, op1=ALU.max)
            nc.vector.tensor_tensor(mxf, mx2[:, 0, :], mx2[:, 1, :],
                                    op=ALU.max)
            nc.gpsimd.partition_all_reduce(bkb, mxf, channels=P,
                                           reduce_op=bass_isa.ReduceOp.max)
            nc.vector.tensor_scalar(bkb, bkb, F32_EPS, 0.25,
                                    op0=ALU.add, op1=ALU.mult)
            if has_bias:
                bsrc = b_d[:]
                bbcast = bass.AP(tensor=bsrc.tensor, offset=bsrc.offset,
                                 ap=[[0, P]] + [list(pair) for pair in bsrc.ap])
                nc.sync.dma_start(biasb, bbcast)

        bx_tiles = {}
        sxt_tiles = {}

        def emit_xpair(j):
            # stage token blocks j and j+1 together.
            # natural-layout stream (gpsimd queue): feeds the per-token bound
            for b in (j, j + 1):
                x_t = xnp.tile([P, D], BF16, tag="x", name="x_t")
                nc.gpsimd.dma_start(x_t, x_d[b * P:(b + 1) * P, :])
                bxq = bxp.tile([P, 1], FP32, tag="bx", name="bxq")
                nc.vector.tensor_reduce(bxq, x_t, axis=AX.X, op=ALU.max,
                                        apply_absolute_value=True)
                nc.vector.tensor_scalar_add(bxq, bxq, F32_EPS)
                bx_tiles[b] = bxq
            # transposed stream through the DMA XBAR, two token blocks per
            # instruction (amortizes the ~12.5us fixed wait each transpose
            # pays): xt[p, kt, t2] = x[jP+t2, kt*P+p] with t2 in [0, 2P).
            # The ucode transpose occupies the issuing engine for its whole
            # transfer, so keeping every transpose on the sync queue also
            # serializes the shared XBAR (two in flight corrupt each other -
            # measured).  ACT signs are split per half so the GEMM, which
            # consumes kt in order, starts on the low half.
            KH = KT // 2
            xt = xtp.tile([P, KT, 2 * P], BF16, tag="xt", name="xt")
            nc.sync.dma_start_transpose(xt, x_d[j * P:(j + 2) * P, :])
            for b in (j, j + 1):
                ts = slice((b - j) * P, (b - j + 1) * P)
                sxt = sxtp.tile([P, KT, P], FP8, tag="sxt", name="sxt")
                nc.scalar.sign(sxt[:, :KH, :], xt[:, :KH, ts], bias=sbias[:])
                nc.scalar.sign(sxt[:, KH:, :], xt[:, KH:, ts], bias=sbias[:])
                sxt_tiles[b] = sxt

        # ---- preamble: the per-feature bounds stream first (they gate the
        # first psum evacuation), block 0/1 staging (so the GEMM can start),
        # then the kernel sign stream (it gates every block's tail), then
        # the remaining lookahead blocks.
        assert PRE % 2 == 0 and TB % 2 == 0
        with tc.tile_pool(name="kload", bufs=3) as kload:
            # tiny throwaway transpose: absorbs the fixed XBAR wake-up wait
            # so block 0's real transpose starts immediately behind it
            warm = const.tile([P, 16], BF16, name="warm")
            nc.sync.dma_start_transpose(warm, x_d[0:16, 0:P])
            # three kernel pairs in flight before anything else on the queue
            for kt in range(0, 6, 2):
                emit_kkick(kload, kt)
            emit_xpair(0)
            for kt in range(0, KT, 2):
                emit_ksign(kt)
                if kt + 6 < KT:
                    emit_kkick(kload, kt + 6)
            emit_bounds_final()
            for j in range(2, PRE, 2):
                emit_xpair(j)

        # ---- main loop over token blocks ----------------------------------
        pm = mybir.MatmulPerfMode.DoubleRow
        for i in range(TB):
            if i + PRE < TB and (i + PRE) % 2 == 0:
                emit_xpair(i + PRE)
            sxt = sxt_tiles.pop(i)
            mm_tiles = [mmps.tile([P, NF], FP32, tag="mm", name=f"mm{fc}")
                        for fc in range(FC)]

            for kt in range(0, KT, 2):
                start = kt == 0
                stop = kt + 2 >= KT
                for fc in range(FC):
                    nc.tensor.matmul(
                        mm_tiles[fc][:],
                        lhsT=sxt[:, kt:kt + 2, :],
                        rhs=sk[:, kt:kt + 2, fc * NF:(fc + 1) * NF],
                        start=start, stop=stop, perf_mode=pm)

            bxq = bx_tiles.pop(i)
            for fc in range(FC):
                sl = slice(fc * NF, (fc + 1) * NF)
                out_c = outp.tile([P, NF], BF16, tag="out", name="out_c")
                # y = (psum * bx[t]) * (0.25*bk[f])
                nc.vector.scalar_tensor_tensor(
                    out_c, mm_tiles[fc][:], bxq, bkb[:, sl],
                    op0=ALU.mult, op1=ALU.mult)
                if has_bias:
                    nc.vector.tensor_tensor(out_c, out_c, biasb[:, sl],
                                            op=ALU.add)
                nc.gpsimd.dma_start(y_d[i * P:(i + 1) * P, sl], out_c)

    if not nc.is_finalized():
        nc.finalize()
    return nc


def _run(x2, ksh_list, bias_list, has_bias, trace=False, NF=512, PRE=6):
    """Compile once and run the SPMD program on all 8 cores."""
    T, D = x2.shape
    F = ksh_list[0].shape[1]
    nc = build_nc(T, D, F, has_bias=has_bias, NF=NF, PRE=PRE)
    x_bf = np.ascontiguousarray(np.asarray(x2).astype(BF16NP))
    in_maps = []
    for c in range(len(ksh_list)):
        k_bf = np.asarray(ksh_list[c]).astype(BF16NP)
        m = {"x_in": x_bf,
             "k_in": np.ascontiguousarray(k_bf)}
        if has_bias:
            m["b_in"] = np.ascontiguousarray(np.asarray(bias_list[c], np.float32))
        in_maps.append(m)
    res = bass_utils.run_bass_kernel_spmd(
        nc, in_maps, core_ids=list(range(len(ksh_list))), trace=trace)
    return res


def kernel(x, kernel, bias):
    x = np.asarray(x, dtype=np.float32)
    k = np.asarray(kernel, dtype=np.float32)
    b = np.asarray(bias, dtype=np.float32)
    B, S, D = x.shape
    F = k.shape[1]
    T = B * S
    FS = F // N_CORES
    x2 = np.ascontiguousarray(x.reshape(T, D))
    has_bias = bool(np.any(b))
    ksh = [np.ascontiguousarray(k[:, c * FS:(c + 1) * FS]) for c in range(N_CORES)]
    bsh = [np.ascontiguousarray(b[c * FS:(c + 1) * FS]) for c in range(N_CORES)]
    res = _run(x2, ksh, bsh, has_bias)
    y = np.concatenate(
        [np.asarray(res.results[c]["y_out"]).astype(np.float32)
         for c in range(N_CORES)], axis=1)
    return np.ascontiguousarray(y.reshape(B, S, F)).astype(np.float32)
